# revision 1
# baseline (speedup 1.0000x reference)
"""Distributed contrastive-loss kernel for Trainium2 (8 NeuronCores).

Strategy (row-sharded, all-gather of normalized features):
  - core c owns rows [1024c, 1024c+1024) of both feature matrices
  - phase 1: normalize local shards (inv-temp folded into cxr rows),
    cast bf16, PE-transpose to [D, rows], AllGather ehr^T (+ diag)
  - phase 2: G-block = cn_shard @ en_full^T via PE (bf16, fp32 PSUM),
    exp fused with row-sums on ACT (accum_out), column partials via
    VE folding + ones-matmul
  - phase 3: AllReduce [colsumexp(8192), sum_nll1(128)], each core
    finishes the scalar loss identically; host reads core 0's output.
"""

import numpy as np

N = 8192
D = 512
NC = 8
SHARD = N // NC  # 1024
P = 128

_cached = None


def _build(N=N, D=D, NC=NC, stop_after="full"):
    SHARD = N // NC
    KC = D // P       # contraction chunks
    MC = SHARD // P   # row chunks per core
    NBIG = N // 1024  # column super-chunks (1024 wide)
    _EHRT_ELEMS = P * KC * SHARD      # bf16 element count of one ehr^T shard
    _EH2 = _EHRT_ELEMS // 2           # same region in f32 slots
    _AGW = _EH2 + SHARD               # f32 AG payload: ehr^T (bitcast) + diag
    _ARW = N + P
    import concourse.bass as bass
    import concourse.tile as tile
    from concourse import bacc, mybir
    from concourse.masks import make_identity

    f32 = mybir.dt.float32
    bf16 = mybir.dt.bfloat16
    AF = mybir.ActivationFunctionType
    ALU = mybir.AluOpType
    X = mybir.AxisListType.X

    nc = bacc.Bacc("TRN2", debug=False, num_devices=NC)

    cxr_d = nc.dram_tensor("cxr", [SHARD, D], f32, kind="ExternalInput").ap()
    ehr_d = nc.dram_tensor("ehr", [SHARD, D], f32, kind="ExternalInput").ap()
    temp_d = nc.dram_tensor("temp", [1, 1], f32, kind="ExternalInput").ap()
    loss_d = nc.dram_tensor("loss", [1, 1], f32, kind="ExternalOutput").ap()

    groups = [list(range(NC))]

    with tile.TileContext(nc) as tc:
        from contextlib import ExitStack

        with ExitStack() as ctx:
            singles = ctx.enter_context(tc.tile_pool(name="singles", bufs=1))
            dram = ctx.enter_context(tc.tile_pool(name="dram", bufs=1, space="DRAM"))

            # persistent SBUF tensors
            cnT = singles.tile([P, KC * SHARD], bf16)    # lhsT: col = k*SHARD + i
            enT = singles.tile([P, KC * N], bf16)        # rhs:  col = k*N + j
            diag = singles.tile([P, MC], f32)            # cos_ii / t, local rows
            diag_all = singles.tile([P, N // P], f32)    # [128, 64] all rows
            rowscr = singles.tile([P, MC * NBIG], f32)   # per (m, n) row sums
            identity_bf = singles.tile([P, P], bf16)
            ones_bf = singles.tile([P, P], bf16)
            ones_f32 = singles.tile([P, 1], f32)

            make_identity(nc, identity_bf[:, :])
            nc.vector.memset(ones_bf[:, :], 1.0)
            nc.vector.memset(ones_f32[:, :], 1.0)

            # temperature -> inv_t, log(inv_t) broadcast to all partitions
            t_sb = singles.tile([P, 1], f32)
            nc.gpsimd.dma_start(out=t_sb[:, :], in_=temp_d.to_broadcast([P, 1]))
            inv_t = singles.tile([P, 1], f32)
            nc.vector.reciprocal(inv_t[:, :], t_sb[:, :])
            log_invt = singles.tile([P, 1], f32)
            nc.scalar.activation(log_invt[:, :], inv_t[:, :], AF.Ln)

            # collective DRAM buffers
            ag_in = dram.tile([_AGW], f32)
            ag_out = dram.tile([NC, _AGW], f32, addr_space="Shared")
            ar_in = dram.tile([_ARW], f32)
            ar_out = dram.tile([_ARW], f32, addr_space="Shared")

            # ---------------- phase 1: normalize + transpose ----------------
            ehrT = singles.tile([P, KC * SHARD], bf16)   # local ehr^T shard

            with ExitStack() as p1:
                natp = p1.enter_context(tc.tile_pool(name="natp", bufs=2))
                nbp = p1.enter_context(tc.tile_pool(name="nbp", bufs=2))
                smallp = p1.enter_context(tc.tile_pool(name="smallp", bufs=4))
                scrp = p1.enter_context(tc.tile_pool(name="scrp", bufs=2))
                ptp = p1.enter_context(
                    tc.tile_pool(name="ptp", bufs=2, space="PSUM")
                )

                for m in range(MC):
                    rs = slice(m * P, (m + 1) * P)
                    xc = natp.tile([P, D], f32, tag="xc")
                    xe = natp.tile([P, D], f32, tag="xe")
                    nc.sync.dma_start(out=xc[:, :], in_=cxr_d[rs, :])
                    nc.sync.dma_start(out=xe[:, :], in_=ehr_d[rs, :])

                    xcb = nbp.tile([P, D], bf16, tag="xcb")
                    xeb = nbp.tile([P, D], bf16, tag="xeb")

                    for xin, xout, is_cxr in ((xc, xcb, True), (xe, xeb, False)):
                        sq = scrp.tile([P, D], bf16, tag="sq")
                        ssq = smallp.tile([P, 1], f32, tag="ssq")
                        nc.scalar.activation(
                            sq[:, :], xin[:, :], AF.Square, accum_out=ssq[:, :]
                        )
                        lssq = smallp.tile([P, 1], f32, tag="lssq")
                        nc.scalar.activation(lssq[:, :], ssq[:, :], AF.Ln)
                        inv = smallp.tile([P, 1], f32, tag="inv")
                        # inv = exp(-0.5*ln(ssq) + [ln(1/t) for cxr])
                        nc.scalar.activation(
                            inv[:, :],
                            lssq[:, :],
                            AF.Exp,
                            scale=-0.5,
                            bias=(log_invt[:, :] if is_cxr else 0.0),
                        )
                        nc.vector.tensor_scalar_mul(xout[:, :], xin[:, :], inv[:, :])

                    # diag_m = sum_d xcb*xeb  (already has 1/t folded in)
                    dscr = scrp.tile([P, D], bf16, tag="dscr")
                    nc.vector.tensor_mul(dscr[:, :], xcb[:, :], xeb[:, :])
                    nc.vector.reduce_sum(diag[:, m : m + 1], dscr[:, :], axis=X)

                    # PE transposes: [128 rows, 128 d] -> [128 d, 128 rows]
                    for xb, dstT in ((xcb, cnT), (xeb, ehrT)):
                        pt = ptp.tile([P, KC * P], bf16, space="PSUM", tag="pt")
                        for k in range(KC):
                            nc.tensor.transpose(
                                pt[:, k * P : (k + 1) * P],
                                xb[:, k * P : (k + 1) * P],
                                identity_bf[:, :],
                            )
                        dst = dstT[:, :].rearrange("p (k i) -> p k i", k=KC)[
                            :, :, m * P : (m + 1) * P
                        ]
                        nc.vector.tensor_copy(
                            out=dst, in_=pt[:, :].rearrange("p (k i) -> p k i", k=KC)
                        )

            def _probe_out(aps):
                """Reduce the given tensors into loss_d (keeps them live)."""
                pv = singles.tile([1, 1], f32, name=f"pv_{len(aps)}")
                first = True
                for i, a in enumerate(aps):
                    r = singles.tile([P, 1], f32, name=f"pr_{i}")
                    nc.vector.reduce_sum(
                        r[: a.shape[0], :], a, axis=mybir.AxisListType.X
                    )
                    if first:
                        nc.vector.tensor_copy(out=pv[:, :], in_=r[0:1, :])
                        first = False
                    else:
                        nc.vector.tensor_add(pv[:, :], pv[:, :], r[0:1, :])
                nc.sync.dma_start(out=loss_d[:, :], in_=pv[:, :])

            done = False
            if stop_after == "phase1":
                _probe_out([cnT[:, :], ehrT[:, :], diag[:, :]])
                done = True

            if not done:
                # ship ehr^T + diag, AllGather
                nc.sync.dma_start(
                    out=ag_in[0:_EH2]
                    .bitcast(bf16)
                    .rearrange("(p c) -> p c", p=P),
                    in_=ehrT[:, :],
                )
                nc.sync.dma_start(
                    out=ag_in[_EH2:_AGW].rearrange("(p m) -> p m", p=P),
                    in_=diag[:, :],
                )
                nc.gpsimd.collective_compute(
                    "AllGather",
                    ALU.bypass,
                    replica_groups=groups,
                    ins=[ag_in[:]],
                    outs=[ag_out[:, :]],
                )

                # gather back: enT[:, k*N + c*SHARD + i] = ag_out[c][p, k, i]
                for c in range(NC):
                    src = (
                        ag_out[c, 0:_EH2]
                        .bitcast(bf16)
                        .rearrange("(p k i) -> p k i", p=P, k=KC)
                    )
                    dst = enT[:, :].rearrange("p (k j) -> p k j", k=KC)[
                        :, :, c * SHARD : (c + 1) * SHARD
                    ]
                    nc.sync.dma_start(out=dst, in_=src)
                    dsrc = ag_out[c, _EH2:_AGW].rearrange("(p m) -> p m", p=P)
                    nc.sync.dma_start(
                        out=diag_all[:, c * MC : (c + 1) * MC], in_=dsrc
                    )

                if stop_after == "ag":
                    _probe_out([enT[:, :], diag_all[:, :]])
                    done = True

            if not done:
                # ------------- phase 2: main similarity block -------------
                stage = singles.tile([P, N // P + 1], f32)

                with ExitStack() as p2:
                    pmp = p2.enter_context(
                        tc.tile_pool(name="pmp", bufs=2, space="PSUM")
                    )
                    pcp = p2.enter_context(
                        tc.tile_pool(name="pcp", bufs=1, space="PSUM")
                    )
                    expp = p2.enter_context(tc.tile_pool(name="expp", bufs=3))
                    accp = p2.enter_context(tc.tile_pool(name="accp", bufs=2))

                    colT = pcp.tile([P, N // P], f32, space="PSUM")

                    for n in range(NBIG):
                        acc = accp.tile([P, 1024], bf16, tag="acc")
                        for m in range(MC):
                            pm = pmp.tile([P, 1024], f32, space="PSUM", tag="pm")
                            for h in range(2):
                                cs = slice(h * 512, (h + 1) * 512)
                                for k in range(KC):
                                    nc.tensor.matmul(
                                        pm[:, cs],
                                        lhsT=cnT[
                                            :,
                                            k * SHARD + m * P : k * SHARD
                                            + (m + 1) * P,
                                        ],
                                        rhs=enT[
                                            :,
                                            k * N + n * 1024 + h * 512 : k * N
                                            + n * 1024
                                            + (h + 1) * 512,
                                        ],
                                        start=(k == 0),
                                        stop=(k == KC - 1),
                                    )
                            if m == 0:
                                et = acc
                            else:
                                et = expp.tile([P, 1024], bf16, tag="et")
                            nc.scalar.activation(
                                et[:, :],
                                pm[:, :],
                                AF.Exp,
                                accum_out=rowscr[
                                    :, m * NBIG + n : m * NBIG + n + 1
                                ],
                            )
                            if m > 0:
                                nc.vector.tensor_add(acc[:, :], acc[:, :], et[:, :])

                        # column partials onto partitions:
                        # colT[:, n*8+t] = acc[:, 128t:128t+128]^T @ ones
                        for t in range(8):
                            nc.tensor.matmul(
                                colT[:, n * 8 + t : n * 8 + t + 1],
                                lhsT=acc[:, t * P : (t + 1) * P],
                                rhs=ones_bf[:, 0:1],
                                start=True,
                                stop=True,
                            )

                    nc.vector.tensor_copy(
                        out=stage[:, 0 : N // P], in_=colT[:, :]
                    )

                if stop_after == "mm":
                    _probe_out([rowscr[:, :], stage[:, :]])
                    done = True

            if not done:
                # --------- phase 3: local nll1, AllReduce, finish ---------
                rowsum = singles.tile([P, MC], f32)
                for m in range(MC):
                    nc.vector.reduce_sum(
                        rowsum[:, m : m + 1],
                        rowscr[:, m * NBIG : (m + 1) * NBIG],
                        axis=X,
                    )
                expd = singles.tile([P, MC], f32)
                nc.scalar.activation(expd[:, :], diag[:, :], AF.Exp)
                rs_ns = singles.tile([P, MC], f32)
                nc.vector.tensor_sub(rs_ns[:, :], rowsum[:, :], expd[:, :])
                lse1 = singles.tile([P, MC], f32)
                nc.scalar.activation(lse1[:, :], rs_ns[:, :], AF.Ln)
                nll1 = singles.tile([P, MC], f32)
                nc.vector.tensor_sub(nll1[:, :], diag[:, :], lse1[:, :])
                nc.vector.reduce_sum(
                    stage[:, N // P : N // P + 1], nll1[:, :], axis=X
                )
                nc.sync.dma_start(
                    out=ar_in[0:_ARW].rearrange("(p w) -> p w", p=P),
                    in_=stage[:, :],
                )

                nc.gpsimd.collective_compute(
                    "AllReduce",
                    ALU.add,
                    replica_groups=groups,
                    ins=[ar_in[:]],
                    outs=[ar_out[:]],
                )

                arback = singles.tile([P, N // P + 1], f32)
                nc.sync.dma_start(
                    out=arback[:, :],
                    in_=ar_out[0:_ARW].rearrange("(p w) -> p w", p=P),
                )

                expd_all = singles.tile([P, N // P], f32)
                nc.scalar.activation(expd_all[:, :], diag_all[:, :], AF.Exp)
                cs_ns = singles.tile([P, N // P], f32)
                nc.vector.tensor_sub(
                    cs_ns[:, :], arback[:, 0 : N // P], expd_all[:, :]
                )
                lse2 = singles.tile([P, N // P], f32)
                nc.scalar.activation(lse2[:, :], cs_ns[:, :], AF.Ln)
                nll2 = singles.tile([P, N // P], f32)
                nc.vector.tensor_sub(nll2[:, :], diag_all[:, :], lse2[:, :])
                t2 = singles.tile([P, 1], f32)
                nc.vector.reduce_sum(t2[:, :], nll2[:, :], axis=X)
                # fold in the AllReduced per-partition nll1 sums
                tfin = singles.tile([P, 1], f32)
                nc.vector.tensor_add(
                    tfin[:, :], t2[:, :], arback[:, N // P : N // P + 1]
                )

                with tc.tile_pool(name="psfin", bufs=1, space="PSUM") as psfin:
                    s2ps = psfin.tile([1, 1], f32, space="PSUM")
                    nc.tensor.matmul(
                        s2ps[:, :],
                        lhsT=ones_f32[:, 0:1],
                        rhs=tfin[:, :],
                        start=True,
                        stop=True,
                    )
                    tot = singles.tile([1, 1], f32)
                    nc.vector.tensor_copy(out=tot[:, :], in_=s2ps[:, :])

                out_sb = singles.tile([1, 1], f32)
                nc.vector.tensor_scalar_mul(out_sb[:, :], tot[:, :], -1.0 / N)
                nc.sync.dma_start(out=loss_d[:, :], in_=out_sb[:, :])

    nc.compile()
    return nc


def _get_nc():
    global _cached
    if _cached is None:
        _cached = _build()
    return _cached


def _make_in_maps(cxr_feats, ehr_feats, temperature):
    cxr = np.ascontiguousarray(np.asarray(cxr_feats, dtype=np.float32))
    ehr = np.ascontiguousarray(np.asarray(ehr_feats, dtype=np.float32))
    t = np.asarray(temperature, dtype=np.float32).reshape(1, 1)
    in_maps = []
    for c in range(NC):
        sl = slice(c * SHARD, (c + 1) * SHARD)
        in_maps.append(
            {
                "cxr": np.ascontiguousarray(cxr[sl]),
                "ehr": np.ascontiguousarray(ehr[sl]),
                "temp": t,
            }
        )
    return in_maps


def run(cxr_feats, ehr_feats, temperature, trace=False):
    """Returns (loss_scalar, BassKernelResults)."""
    from concourse import bass_utils

    nc = _get_nc()
    in_maps = _make_in_maps(cxr_feats, ehr_feats, temperature)
    res = bass_utils.run_bass_kernel_spmd(
        nc, in_maps, core_ids=list(range(NC)), trace=trace
    )
    loss = np.float32(np.asarray(res.results[0]["loss"]).reshape(-1)[0])
    return np.asarray(loss, dtype=np.float32).reshape(()), res


def kernel(cxr_feats, ehr_feats, temperature):
    loss, _ = run(cxr_feats, ehr_feats, temperature, trace=False)
    return loss



# revision 9
# speedup vs baseline: 1.1047x; 1.1047x over previous
"""Distributed contrastive-loss kernel for Trainium2 (8 NeuronCores).

Strategy (row-sharded, all-gather of normalized features):
  - core c owns rows [1024c, 1024c+1024) of both feature matrices
  - phase 1a: normalize ehr shard (scaled x16, bf16), PE-transpose to
    [D, rows], fire AllGather #1 (ehr^T only) as early as possible
  - phase 1b: normalize cxr shard, transpose, diag_m = <cn,en> (x256),
    fire tiny AllGather #2 (diag)
  - phase 2: G-block = cn_shard @ en_full^T via PE (bf16, fp32 PSUM),
    exp fused with row-sums on ACT (scale operand folds 1/(256 t)),
    column partials via VE folding + ones-matmul
  - phase 3: AllReduce [colsumexp(8192), sum_nll1(128)], each core
    finishes the scalar loss identically; host reads core 0's output.
"""

import numpy as np

N = 8192
D = 512
NC = 8
SHARD = N // NC  # 1024
P = 128

_cached = None


def _build(N=N, D=D, NC=NC, stop_after="full"):
    import os
    P1L = int(os.environ.get("P1_LEVEL", "9"))
    SHARD = N // NC
    KC = D // P       # contraction chunks
    MC = SHARD // P   # row chunks per core
    NBIG = N // 1024  # column super-chunks (1024 wide)
    _EHRT_ELEMS = P * KC * SHARD      # bf16 element count of one ehr^T shard
    _EH2 = _EHRT_ELEMS // 2           # same region in f32 slots
    _ARW = N + P
    import concourse.bass as bass
    import concourse.tile as tile
    from concourse import bacc, mybir
    from concourse.masks import make_identity

    f32 = mybir.dt.float32
    bf16 = mybir.dt.bfloat16
    AF = mybir.ActivationFunctionType
    ALU = mybir.AluOpType
    X = mybir.AxisListType.X

    nc = bacc.Bacc("TRN2", debug=False, num_devices=NC)

    cxr_d = nc.dram_tensor("cxr", [SHARD, D], f32, kind="ExternalInput").ap()
    ehr_d = nc.dram_tensor("ehr", [SHARD, D], f32, kind="ExternalInput").ap()
    temp_d = nc.dram_tensor("temp", [1, 1], f32, kind="ExternalInput").ap()
    loss_d = nc.dram_tensor("loss", [1, 1], f32, kind="ExternalOutput").ap()

    groups = [list(range(NC))]

    with tile.TileContext(nc) as tc:
        from contextlib import ExitStack

        with ExitStack() as ctx:
            singles = ctx.enter_context(tc.tile_pool(name="singles", bufs=1))
            dram = ctx.enter_context(tc.tile_pool(name="dram", bufs=1, space="DRAM"))

            # persistent SBUF tensors
            cnT = singles.tile([P, KC * SHARD], bf16)    # lhsT: col = k*SHARD + i
            enT = singles.tile([P, KC * N], bf16)        # rhs:  col = k*N + j
            ehrT = singles.tile([P, KC * SHARD], bf16)   # local ehr^T shard
            diag = singles.tile([P, MC], f32)            # 256*cos_ii, local rows
            diag_all = singles.tile([P, N // P], f32)    # [128, 64] all rows
            rowscr = singles.tile([P, MC * NBIG], f32)   # per (m, n) row sums
            identity_bf = singles.tile([P, P], bf16)
            ones_bf = singles.tile([P, P], bf16)
            ones_f32 = singles.tile([P, 1], f32)
            xc = singles.tile([P, MC * D], f32)          # staged cxr (row-chunked)
            xe = singles.tile([P, MC * D], f32)          # staged ehr
            xebs = singles.tile([P, MC * D], bf16)       # normalized ehr (for diag)
            dd = singles.tile([P, MC], f32)              # diag/t
            dd_all = singles.tile([P, N // P], f32)
            expd = singles.tile([P, MC], f32)            # exp(diag/t)
            expd_all = singles.tile([P, N // P], f32)

            make_identity(nc, identity_bf[:, :])
            nc.vector.memset(ones_bf[:, :], 1.0)
            nc.vector.memset(ones_f32[:, :], 1.0)

            # temperature -> inv_t/256 (features are scaled x16 each, so the
            # raw PE output is 256*cos; exp(scale*x) folds both corrections)
            t_sb = singles.tile([P, 1], f32)
            nc.gpsimd.dma_start(out=t_sb[:, :], in_=temp_d.to_broadcast([P, 1]))
            inv_t = singles.tile([P, 1], f32)
            nc.vector.reciprocal(inv_t[:, :], t_sb[:, :])
            invt256 = singles.tile([P, 1], f32)
            nc.vector.tensor_scalar_mul(invt256[:, :], inv_t[:, :], 1.0 / 256.0)

            # collective DRAM buffers
            ag_in = dram.tile([_EH2], f32)
            ag_out = dram.tile([NC, _EH2], f32, addr_space="Shared")
            agd_in = dram.tile([P * MC], f32)
            agd_out = dram.tile([NC, P * MC], f32, addr_space="Shared")
            ar_in = dram.tile([_ARW], f32)
            ar_out = dram.tile([_ARW], f32, addr_space="Shared")

            # per-chunk input DMAs, spread across queues
            for m in range(MC):
                rs = slice(m * P, (m + 1) * P)
                nc.sync.dma_start(
                    out=xe[:, m * D : (m + 1) * D], in_=ehr_d[rs, :]
                )
            for m in range(MC):
                rs = slice(m * P, (m + 1) * P)
                nc.gpsimd.dma_start(
                    out=xc[:, m * D : (m + 1) * D], in_=cxr_d[rs, :]
                )

            # ---------------- phase 1: normalize + transpose ----------------
            def norm_chunk(m, xin_all, pools, out_bf):
                """out_bf = 16 * x / ||x|| for row-chunk m, as bf16."""
                natp, smallp, scrp = pools
                xin = xin_all[:, m * D : (m + 1) * D]
                sq = scrp.tile([P, D], bf16, tag="sq")
                ssq = smallp.tile([P, 1], f32, tag="ssq")
                nc.scalar.activation(
                    sq[:, :], xin, AF.Square, accum_out=ssq[:, :]
                )
                nrm = smallp.tile([P, 1], f32, tag="nrm")
                # ||x||/16 = sqrt(ssq/256)
                nc.scalar.activation(nrm[:, :], ssq[:, :], AF.Sqrt, scale=1.0 / 256.0)
                inv = smallp.tile([P, 1], f32, tag="inv")
                nc.vector.reciprocal(inv[:, :], nrm[:, :])
                nc.vector.tensor_scalar_mul(out_bf, xin, inv[:, :])

            def transpose_chunk(m, xb, dstT, ptp):
                pt = ptp.tile([P, KC * P], bf16, space="PSUM", tag="pt")
                for k in range(KC):
                    nc.tensor.transpose(
                        pt[:, k * P : (k + 1) * P],
                        xb[:, k * P : (k + 1) * P],
                        identity_bf[:, :],
                    )
                dst = dstT[:, :].rearrange("p (k i) -> p k i", k=KC)[
                    :, :, m * P : (m + 1) * P
                ]
                nc.vector.tensor_copy(
                    out=dst, in_=pt[:, :].rearrange("p (k i) -> p k i", k=KC)
                )

            with ExitStack() as p1:
                natp = p1.enter_context(tc.tile_pool(name="natp", bufs=2))
                smallp = p1.enter_context(tc.tile_pool(name="smallp", bufs=4))
                scrp = p1.enter_context(tc.tile_pool(name="scrp", bufs=2))
                ptp = p1.enter_context(
                    tc.tile_pool(name="ptp", bufs=2, space="PSUM")
                )
                pools = (natp, smallp, scrp)

                # 1a: ehr first so the big AllGather can fire early
                for m in range(MC):
                    xeb = xebs[:, m * D : (m + 1) * D]
                    if P1L >= 1:
                        norm_chunk(m, xe, pools, xeb)
                    if P1L >= 2:
                        transpose_chunk(m, xeb, ehrT, ptp)

                # ship ehr^T, AllGather #1 (the big one)
                if P1L >= 3:
                    nc.sync.dma_start(
                        out=ag_in[0:_EH2]
                        .bitcast(bf16)
                        .rearrange("(p c) -> p c", p=P),
                        in_=ehrT[:, :],
                    )
                    nc.gpsimd.collective_compute(
                        "AllGather",
                        ALU.bypass,
                        replica_groups=groups,
                        ins=[ag_in[:]],
                        outs=[ag_out[:, :]],
                    )

                # 1b: cxr + diag
                for m in range(MC):
                    xcb = natp.tile([P, D], bf16, tag="xcb")
                    if P1L >= 1:
                        norm_chunk(m, xc, pools, xcb[:, :])
                    if P1L >= 2:
                        transpose_chunk(m, xcb, cnT, ptp)
                    if P1L >= 4:
                        dscr = scrp.tile([P, D], bf16, tag="dscr")
                        nc.vector.tensor_mul(
                            dscr[:, :], xcb[:, :], xebs[:, m * D : (m + 1) * D]
                        )
                        nc.vector.reduce_sum(
                            diag[:, m : m + 1], dscr[:, :], axis=X
                        )

                # ship diag, AllGather #2 (tiny)
                if P1L >= 5:
                    nc.sync.dma_start(
                        out=agd_in[:].rearrange("(p m) -> p m", p=P),
                        in_=diag[:, :],
                    )
                    nc.gpsimd.collective_compute(
                        "AllGather",
                        ALU.bypass,
                        replica_groups=groups,
                        ins=[agd_in[:]],
                        outs=[agd_out[:, :]],
                    )

            def _probe_out(aps):
                """Reduce the given tensors into loss_d (keeps them live)."""
                pv = singles.tile([1, 1], f32, name=f"pv_{len(aps)}")
                first = True
                for i, a in enumerate(aps):
                    r = singles.tile([P, 1], f32, name=f"pr_{i}")
                    nc.vector.reduce_sum(
                        r[: a.shape[0], :], a, axis=mybir.AxisListType.X
                    )
                    if first:
                        nc.vector.tensor_copy(out=pv[:, :], in_=r[0:1, :])
                        first = False
                    else:
                        nc.vector.tensor_add(pv[:, :], pv[:, :], r[0:1, :])
                nc.sync.dma_start(out=loss_d[:, :], in_=pv[:, :])

            done = False
            if stop_after == "phase1":
                aps = [xc[:, :], xe[:, :]]
                if P1L >= 1:
                    aps.append(xebs[:, :])
                if P1L >= 2:
                    aps += [cnT[:, :], ehrT[:, :]]
                if P1L >= 4:
                    aps.append(diag[:, :])
                _probe_out(aps)
                done = True

            if not done:
                # gather back: enT[:, k*N + c*SHARD + i] = ag_out[c][p, k, i]
                issuers = [nc.sync, nc.scalar, nc.gpsimd]
                for c in range(NC):
                    src = (
                        ag_out[c, 0:_EH2]
                        .bitcast(bf16)
                        .rearrange("(p k i) -> p k i", p=P, k=KC)
                    )
                    dst = enT[:, :].rearrange("p (k j) -> p k j", k=KC)[
                        :, :, c * SHARD : (c + 1) * SHARD
                    ]
                    issuers[c % 3].dma_start(out=dst, in_=src)
                for c in range(NC):
                    dsrc = agd_out[c, :].rearrange("(p m) -> p m", p=P)
                    issuers[(c + 1) % 3].dma_start(
                        out=diag_all[:, c * MC : (c + 1) * MC], in_=dsrc
                    )

                if stop_after == "ag":
                    _probe_out([enT[:, :], diag_all[:, :]])
                    done = True

            if not done:
                # ------------- phase 2: main similarity block -------------
                stage = singles.tile([P, N // P + 1], f32)

                with ExitStack() as p2:
                    pmp = p2.enter_context(
                        tc.tile_pool(name="pmp", bufs=2, space="PSUM")
                    )
                    pcp = p2.enter_context(
                        tc.tile_pool(name="pcp", bufs=1, space="PSUM")
                    )
                    expp = p2.enter_context(tc.tile_pool(name="expp", bufs=3))
                    accp = p2.enter_context(tc.tile_pool(name="accp", bufs=2))

                    colT = pcp.tile([P, N // P], f32, space="PSUM")

                    for n in range(NBIG):
                        acc = accp.tile([P, 1024], bf16, tag="acc")
                        for m in range(MC):
                            pm = pmp.tile([P, 1024], f32, space="PSUM", tag="pm")
                            for h in range(2):
                                cs = slice(h * 512, (h + 1) * 512)
                                for k in range(KC):
                                    nc.tensor.matmul(
                                        pm[:, cs],
                                        lhsT=cnT[
                                            :,
                                            k * SHARD + m * P : k * SHARD
                                            + (m + 1) * P,
                                        ],
                                        rhs=enT[
                                            :,
                                            k * N + n * 1024 + h * 512 : k * N
                                            + n * 1024
                                            + (h + 1) * 512,
                                        ],
                                        start=(k == 0),
                                        stop=(k == KC - 1),
                                    )
                            if m == 0:
                                et = acc
                            else:
                                et = expp.tile([P, 1024], bf16, tag="et")
                            # et = exp(cos/t): scale folds 1/(256 t)
                            nc.scalar.activation(
                                et[:, :],
                                pm[:, :],
                                AF.Exp,
                                scale=invt256[:, :],
                                accum_out=rowscr[
                                    :, m * NBIG + n : m * NBIG + n + 1
                                ],
                            )
                            if m > 0:
                                nc.vector.tensor_add(acc[:, :], acc[:, :], et[:, :])

                        # column partials onto partitions:
                        # colT[:, n*8+t] = acc[:, 128t:128t+128]^T @ ones
                        for t in range(8):
                            nc.tensor.matmul(
                                colT[:, n * 8 + t : n * 8 + t + 1],
                                lhsT=acc[:, t * P : (t + 1) * P],
                                rhs=ones_bf[:, 0:1],
                                start=True,
                                stop=True,
                            )

                        if n == 0:
                            # precompute exp(diag/t) for both phase-3 legs now
                            # (keeps the serial tail Ln-only; Exp table is hot)
                            nc.vector.tensor_scalar_mul(
                                dd[:, :], diag[:, :], invt256[:, :]
                            )
                            nc.vector.tensor_scalar_mul(
                                dd_all[:, :], diag_all[:, :], invt256[:, :]
                            )
                            nc.scalar.activation(expd[:, :], dd[:, :], AF.Exp)
                            nc.scalar.activation(
                                expd_all[:, :], dd_all[:, :], AF.Exp
                            )

                    nc.vector.tensor_copy(
                        out=stage[:, 0 : N // P], in_=colT[:, :]
                    )

                if stop_after == "mm":
                    _probe_out([rowscr[:, :], stage[:, :]])
                    done = True

            if not done:
                # --------- phase 3: local nll1, AllReduce, finish ---------
                rowsum = singles.tile([P, MC], f32)
                nc.vector.reduce_sum(
                    rowsum[:, :],
                    rowscr[:, :].rearrange("p (m n) -> p m n", m=MC),
                    axis=X,
                )
                rs_ns = singles.tile([P, MC], f32)
                nc.vector.tensor_sub(rs_ns[:, :], rowsum[:, :], expd[:, :])
                lse1 = singles.tile([P, MC], f32)
                nc.scalar.activation(lse1[:, :], rs_ns[:, :], AF.Ln)
                nll1 = singles.tile([P, MC], f32)
                nc.vector.tensor_sub(nll1[:, :], dd[:, :], lse1[:, :])
                nc.vector.reduce_sum(
                    stage[:, N // P : N // P + 1], nll1[:, :], axis=X
                )
                nc.sync.dma_start(
                    out=ar_in[0:_ARW].rearrange("(p w) -> p w", p=P),
                    in_=stage[:, :],
                )

                nc.gpsimd.collective_compute(
                    "AllReduce",
                    ALU.add,
                    replica_groups=groups,
                    ins=[ar_in[:]],
                    outs=[ar_out[:]],
                )

                arback = singles.tile([P, N // P + 1], f32)
                nc.sync.dma_start(
                    out=arback[:, :],
                    in_=ar_out[0:_ARW].rearrange("(p w) -> p w", p=P),
                )

                cs_ns = singles.tile([P, N // P], f32)
                nc.vector.tensor_sub(
                    cs_ns[:, :], arback[:, 0 : N // P], expd_all[:, :]
                )
                lse2 = singles.tile([P, N // P], f32)
                nc.scalar.activation(lse2[:, :], cs_ns[:, :], AF.Ln)
                nll2 = singles.tile([P, N // P], f32)
                nc.vector.tensor_sub(nll2[:, :], dd_all[:, :], lse2[:, :])
                t2 = singles.tile([P, 1], f32)
                nc.vector.reduce_sum(t2[:, :], nll2[:, :], axis=X)
                # fold in the AllReduced per-partition nll1 sums
                tfin = singles.tile([P, 1], f32)
                nc.vector.tensor_add(
                    tfin[:, :], t2[:, :], arback[:, N // P : N // P + 1]
                )

                with tc.tile_pool(name="psfin", bufs=1, space="PSUM") as psfin:
                    s2ps = psfin.tile([1, 1], f32, space="PSUM")
                    nc.tensor.matmul(
                        s2ps[:, :],
                        lhsT=ones_f32[:, 0:1],
                        rhs=tfin[:, :],
                        start=True,
                        stop=True,
                    )
                    tot = singles.tile([1, 1], f32)
                    nc.vector.tensor_copy(out=tot[:, :], in_=s2ps[:, :])

                out_sb = singles.tile([1, 1], f32)
                nc.vector.tensor_scalar_mul(out_sb[:, :], tot[:, :], -1.0 / N)
                nc.sync.dma_start(out=loss_d[:, :], in_=out_sb[:, :])

    nc.compile()
    return nc


def _get_nc():
    global _cached
    if _cached is None:
        _cached = _build()
    return _cached


def _make_in_maps(cxr_feats, ehr_feats, temperature):
    cxr = np.ascontiguousarray(np.asarray(cxr_feats, dtype=np.float32))
    ehr = np.ascontiguousarray(np.asarray(ehr_feats, dtype=np.float32))
    t = np.asarray(temperature, dtype=np.float32).reshape(1, 1)
    in_maps = []
    for c in range(NC):
        sl = slice(c * SHARD, (c + 1) * SHARD)
        in_maps.append(
            {
                "cxr": np.ascontiguousarray(cxr[sl]),
                "ehr": np.ascontiguousarray(ehr[sl]),
                "temp": t,
            }
        )
    return in_maps


def run(cxr_feats, ehr_feats, temperature, trace=False):
    """Returns (loss_scalar, BassKernelResults)."""
    from concourse import bass_utils

    nc = _get_nc()
    in_maps = _make_in_maps(cxr_feats, ehr_feats, temperature)
    res = bass_utils.run_bass_kernel_spmd(
        nc, in_maps, core_ids=list(range(NC)), trace=trace
    )
    loss = np.float32(np.asarray(res.results[0]["loss"]).reshape(-1)[0])
    return np.asarray(loss, dtype=np.float32).reshape(()), res


def kernel(cxr_feats, ehr_feats, temperature):
    loss, _ = run(cxr_feats, ehr_feats, temperature, trace=False)
    return loss
